# revision 46
# baseline (speedup 1.0000x reference)
"""Distributed Trainium2 Bass kernel for nn_Attention_87368224735328.

reference:
    score = einsum("bqd,bkd->bqk", enc_outputs, atten_outputs)   # [B,S1,S2]
    alignment = softmax(score, axis=-1)                          # over S2
    out = einsum("bqk,bqd->bkd", alignment, enc_outputs + enc_residual)

Sharding: 8 cores = (batch b in 0..3) x (S2-half in 0..1). Each core computes
its local [S1, S2/2] score block, local softmax row-stats (max / sum-exp over
its S2 half), exchanges the tiny [S1] stats with its partner core, and runs
the second GEMM fully locally (contraction over S1 is complete on every
core). Output shard: [S2/2, D] -> out[b, half].

The kernel is TensorEngine-bound (fp16, 2 x 131072 PE cycles ~ 109 us at
2.4 GHz), so the schedule keeps PE as gapless as possible across the
GEMM1 -> GEMM2 seam and the softmax-stats exchange (PE p-state drops on
idle: a multi-us gap costs ~6.5 us extra re-ramp on top of the gap):

- Stats travel via a send-side-masked ReduceScatter: each core writes its
  (-m, z) stats into its PARTNER's chunk slot (one-hot sel mask) and zeros
  elsewhere, so the summed chunk a core receives IS the partner's stats --
  no rank-indexed gather or select on the post-collective path, and the
  receive is one small contiguous DMA. Two exchanges: A covers q tiles
  [0,8) and completes its merge just before GEMM1 ends (~59.5 us); Z covers
  [8,16), is issued after the last GEMM1 tile, and completes under GEMM2's
  phase 0.
- GEMM2 is split into phase 0 (q tiles [0,8), dependent only on exchange A)
  and phase 1 ([8,16), dependent on Z). ALL eight ki output tiles run their
  phase-0 accumulation first (~27 us of PE work, covering Z's collective
  latency), spilling the partial [128, D] sums to SBUF through the scalar
  engine (Copy shares the Exp activation table -> no table-switch cost).
  Phase 1 reruns all ki with fresh PSUM groups and the DVE adds
  partial + psum on the way out; the last ki's two column halves use
  separate PSUM tiles so the first half's add + store pipelines under the
  second half's final matmuls.
- The merge is one-exp: cs = 1/(z0 + exp(n0 - n1) * z1) (exp overflow
  saturates to cs = 0, the correct limit), so the post-collective chain is
  a receive DMA + sub (DVE) + exp (Act) + mul/add/recip (DVE).
- Queue discipline: exchange DMAs ride the sync queue, which parks them
  early with monotonically-increasing waits (sendA, recvA, sendZ, recvZ) so
  the compiler's issue-hoisting cannot invert them; exchange A's merge is
  emitted after tile 14's reduce so that reduce runs at its natural time
  and only tile 15's reduce/exp trail the merge (they gate exchange Z's
  send); the A v-scales are split DVE/Act in consumption order so their
  joint rate just covers GEMM2 phase 0's operand consumption. The last
  ki's phase-1 accumulation uses four independent 256-column PSUM tiles so
  each quarter's add + store pipelines under the next quarter's matmuls.

Precision: fp16 operands on the TensorEngine (full rate, ~16x finer mantissa
than bf16 -- needed because the scores have std ~32 so softmax is nearly
one-hot and bf16 score error flips argmaxes). Accumulation is f32 in PSUM,
stats/softmax math in f32. Measured end-to-end rel err vs f32 reference ~1.6e-3.
"""

import numpy as np

from concourse import bacc, mybir, tile
from concourse.bass_utils import run_bass_kernel_spmd

B, S, D = 4, 2048, 1024
S2L = S // 2          # local S2 columns per core
NQT = S // 128        # 16 q tiles (S1)
NDC = D // 128        # 8 contraction chunks for GEMM1
NKB = S2L // 512      # 2 PSUM blocks of 512 for GEMM1
NKT = S2L // 128      # 8 output k tiles for GEMM2
SPLIT = 8             # q-tile boundary between exchange A and exchange Z
FP16 = mybir.dt.float16
F32 = mybir.dt.float32
N_CORES = 8
RG8 = [[0, 1, 2, 3, 4, 5, 6, 7]]
RGP = [[0, 1], [2, 3], [4, 5], [6, 7]]


def _emit_stats_send(nc, P, DR, sel2_sb, negm, zloc, lo, hi, tag,
                     use_collective, send_eng=None):
    send_eng = send_eng or nc.sync
    """Ship local (-m, z) for q tiles [lo, hi) to the partner core via a
    send-side-masked pairwise ReduceScatter: stats_in has TWO 128-row slots
    (ReduceScatter distributes along partitions in p_dim/group slabs); each
    core writes its stats into the slot its PARTNER will receive (one-hot
    sel2 mask on parity) and zeros into its own, so the summed slot a core
    receives IS the partner's stats -- no rank-indexed gather or select on
    the post-collective path. Returns the [128, 2n] DRAM output."""
    n = hi - lo
    # mask directly from negm/zloc (no staging copy): the negm halves only
    # wait on the last reduce, the zloc halves on the last exp
    msk = P.tile([128, 4 * n], F32, tag=f"msk{tag}", name=f"msk{tag}")
    for r in range(2):
        nc.vector.tensor_scalar_mul(
            out=msk[:, r * 2 * n:r * 2 * n + n], in0=negm[:, lo:hi],
            scalar1=sel2_sb[:, r:r + 1])
        nc.vector.tensor_scalar_mul(
            out=msk[:, r * 2 * n + n:(r + 1) * 2 * n], in0=zloc[:, lo:hi],
            scalar1=sel2_sb[:, r:r + 1])
    stats_in = DR.tile([2 * 128, 2 * n], F32, tag=f"si{tag}",
                       name=f"stats_in{tag}")
    stats_out = DR.tile([128, 2 * n], F32, tag=f"so{tag}",
                        name=f"stats_out{tag}")
    for r in range(2):
        send_eng.dma_start(out=stats_in[r * 128:(r + 1) * 128, :],
                           in_=msk[:, r * 2 * n:(r + 1) * 2 * n])
    if use_collective:
        nc.gpsimd.collective_compute(
            "ReduceScatter", mybir.AluOpType.add,
            replica_groups=RGP,
            ins=[stats_in[:, :].opt()],
            outs=[stats_out[:, :].opt()],
            cc_dim="Partition",
        )
    else:  # debug/sim variant: self-merge via slot sum
        nc.gpsimd.dma_start(out=stats_out[:, :], in_=msk[:, 0:2 * n])
    return stats_out


def _emit_stats_recv_dma(nc, P, stats_out, n, tag):
    """Pull the partner stats in via the sync queue (parked early; its
    waits stay monotone there)."""
    acc = P.tile([128, 2 * n], F32, tag=f"acc{tag}", name=f"acc{tag}")
    nc.sync.dma_start(out=acc[:, :], in_=stats_out[:, :])
    return acc


def _emit_stats_recv(nc, P, stats_out, negm, lo, hi, tag, acc=None):
    """Compute the merge exponent d = n0 - n1 on the DVE (receiving the
    stats first if not already pulled in). Returns (acc, d)."""
    n = hi - lo
    if acc is None:
        acc = _emit_stats_recv_dma(nc, P, stats_out, n, tag)
    # cs = t0/Z_glob = 1/(z0 + exp(n0 - n1) * z1) with n_i = -m_i: one
    # exp, no global max needed. exp overflow (partner max >> local max)
    # saturates to inf -> cs = 0, the correct limit; underflow -> 1/z0.
    d = P.tile([128, n], F32, tag=f"d{tag}", name=f"d{tag}")
    nc.vector.tensor_sub(out=d[:, :], in0=negm[:, lo:hi], in1=acc[:, 0:n])
    return acc, d


def _emit_stats_finish(nc, P, acc, d, zloc, cs, lo, hi, tag):
    """Exp (scalar engine) + Z_glob merge + reciprocal (DVE) -> cs[lo:hi].
    cs = 1 / (z0 + exp(n0 - n1) * z1)."""
    n = hi - lo
    t = P.tile([128, n], F32, tag=f"t{tag}", name=f"t{tag}")
    zg = P.tile([128, n], F32, tag=f"zg{tag}", name=f"zg{tag}")
    nc.scalar.activation(out=t[:, :], in_=d[:, :],
                         func=mybir.ActivationFunctionType.Exp)
    nc.vector.tensor_mul(out=t[:, :], in0=t[:, :], in1=acc[:, n:2 * n])
    nc.vector.tensor_add(out=zg[:, :], in0=t[:, :], in1=zloc[:, lo:hi])
    nc.vector.reciprocal(out=cs[:, lo:hi], in_=zg[:, :])


def _emit_body(nc, tc, pools, qT, kT, enc, res, sel, out, use_collective):
    P, ST, PS, OST, DR = pools

    # ---- persistent SBUF tensors -------------------------------
    qt_sb = [P.tile([128, S], FP16, tag=f"qt{c}", name=f"qt{c}")
             for c in range(NDC)]
    kt_sb = [P.tile([128, S2L], FP16, tag=f"kt{c}", name=f"kt{c}")
             for c in range(NDC)]
    v_sb = [P.tile([128, D], FP16, tag=f"v{i}", name=f"v{i}")
            for i in range(NQT)]
    e_sb = [P.tile([128, S2L], FP16, tag=f"e{i}", name=f"e{i}")
            for i in range(NQT)]
    part = [P.tile([128, D], F32, tag=f"pp{i}", name=f"pp{i}")
            for i in range(NKT)]
    negm = P.tile([128, NQT], F32, tag="negm", name="negm")
    zloc = P.tile([128, NQT], F32, tag="zloc", name="zloc")
    cs = P.tile([128, NQT], F32, tag="cs", name="cs")
    sel2_sb = P.tile([128, 2], F32, tag="sel2", name="sel2_sb")

    # ---- load GEMM1 operands (d on partitions, pre-transposed) --
    # Two HWDGE queues in parallel: kt chunks issue from the (ramp-idle)
    # scalar engine, qt from sync. qt is streamed in two column waves so
    # the ramp tiles' columns [0:512) all land first.
    # kt0 via SWDGE (Pool engine, idle at start), split in halves so the
    # first matmul only waits on the first 512 columns; the scalar engine
    # runs the hoisted ACT table load (~2.7us) first, which must not gate
    # the first matmul, so later kt chunks ride scalar
    nc.gpsimd.dma_start(out=kt_sb[0][:, 0:512], in_=kT[0:128, 0:512])
    nc.gpsimd.dma_start(out=kt_sb[0][:, 512:S2L], in_=kT[0:128, 512:S2L])
    for c in range(NDC):
        if c > 0:
            nc.scalar.dma_start(out=kt_sb[c][:, :],
                                in_=kT[c * 128:(c + 1) * 128, :])
        nc.sync.dma_start(out=qt_sb[c][:, 0:512],
                          in_=qT[c * 128:(c + 1) * 128, 0:512])
    for c in range(NDC):
        nc.sync.dma_start(out=qt_sb[c][:, 512:2048],
                          in_=qT[c * 128:(c + 1) * 128, 512:2048])
    nc.sync.dma_start(out=sel2_sb[:, :], in_=sel)

    # ---- GEMM1 + local softmax stats per q tile ----------------
    RAMP = 4  # first tiles run chunk-major so each arriving chunk feeds 8 MMs
    # staircase: tile qi consumes chunk s-qi at step s, so tile completions
    # stagger and the softmax consumers drain while later tiles finish
    ramp_ps = [PS.tile([128, S2L], F32, tag="ps", name=f"s{qi}")
               for qi in range(RAMP)]
    for s in range(NDC + RAMP - 1):
        for qi in range(RAMP):
            dc = s - qi
            if not 0 <= dc < NDC:
                continue
            for kb in range(NKB):
                nc.tensor.matmul(
                    ramp_ps[qi][:, kb * 512:(kb + 1) * 512],
                    lhsT=qt_sb[dc][:, qi * 128:(qi + 1) * 128],
                    rhs=kt_sb[dc][:, kb * 512:(kb + 1) * 512],
                    start=(dc == 0),
                    stop=(dc == NDC - 1),
                )
    stats_a = None
    for qi in range(NQT):
        if qi < RAMP:
            ps = ramp_ps[qi]
        else:
            ps = PS.tile([128, S2L], F32, tag="ps", name=f"s{qi}")
            for dc in range(NDC):
                for kb in range(NKB):
                    nc.tensor.matmul(
                        ps[:, kb * 512:(kb + 1) * 512],
                        lhsT=qt_sb[dc][:, qi * 128:(qi + 1) * 128],
                        rhs=kt_sb[dc][:, kb * 512:(kb + 1) * 512],
                        start=(dc == 0),
                        stop=(dc == NDC - 1),
                    )
        nc.vector.tensor_reduce(
            out=negm[:, qi:qi + 1], in_=ps[:, :],
            axis=mybir.AxisListType.X, op=mybir.AluOpType.max, negate=True)
        # E = exp(S - m_loc) (fp16), Z_loc = row-sum(E) (f32)
        nc.scalar.activation(
            out=e_sb[qi][:, :], in_=ps[:, :],
            func=mybir.ActivationFunctionType.Exp,
            bias=negm[:, qi:qi + 1], scale=1.0,
            accum_out=zloc[:, qi:qi + 1])

        # overlap: V tile load + add while GEMM1 runs
        enc_t = ST.tile([128, D], FP16, tag="enc", name=f"enc{qi}")
        res_t = ST.tile([128, D], FP16, tag="res", name=f"res{qi}")
        nc.sync.dma_start(out=enc_t[:, :],
                          in_=enc[qi * 128:(qi + 1) * 128, :])
        nc.sync.dma_start(out=res_t[:, :],
                          in_=res[qi * 128:(qi + 1) * 128, :])
        nc.vector.tensor_add(out=v_sb[qi][:, :], in0=enc_t[:, :],
                             in1=res_t[:, :])

        if qi + 1 == SPLIT:
            # exchange A launches as soon as tile SPLIT-1's stats exist; the
            # collective runs under the rest of GEMM1
            stats_a = _emit_stats_send(nc, P, DR, sel2_sb, negm, zloc, 0,
                                       SPLIT, "a", use_collective)
        if qi == NQT - 3:
            # pull exchange A's result in now: the sync queue reaches this
            # park position one tile earlier than the merge needs it
            acc_a = _emit_stats_recv_dma(nc, P, stats_a, SPLIT, "a")
        if qi == NQT - 2:
            # merge A here (after reduce14, which then runs at its natural
            # time): the operand waits can only delay tile 15's reduce/exp,
            # which still clears well before GEMM2 phase 1 needs it
            acc, d_a = _emit_stats_recv(nc, P, stats_a, negm, 0, SPLIT,
                                        "a", acc=acc_a)
            _emit_stats_finish(nc, P, acc, d_a, zloc, cs, 0, SPLIT, "a")
            # v-scales split by column half in need order (GEMM2 phase 0
            # consumes v[qj][:, 0:512] ~1.7us before the second half); the
            # first db1 tiles ride the scalar engine so the DVE frees up
            # ~1.7us sooner for tiles 14/15's reduces, which gate the whole
            # exchange-Z chain
            for qj in range(SPLIT):
                nc.vector.tensor_scalar_mul(
                    out=v_sb[qj][:, 0:512], in0=v_sb[qj][:, 0:512],
                    scalar1=cs[:, qj:qj + 1])
            for qj in range(4):
                nc.vector.tensor_scalar_mul(
                    out=v_sb[qj][:, 512:D], in0=v_sb[qj][:, 512:D],
                    scalar1=cs[:, qj:qj + 1])
            for qj in range(4, SPLIT):
                nc.scalar.mul(out=v_sb[qj][:, 512:D],
                              in_=v_sb[qj][:, 512:D],
                              mul=cs[:, qj:qj + 1])

    # final exchange Z: send + receive + Pool merge prologue now; the
    # exp/merge tail is emitted mid-GEMM2 (after the first spill copies) so
    # the Act queue's PSUM-recycling copies are never stuck behind its
    # operand wait
    stats_z = _emit_stats_send(nc, P, DR, sel2_sb, negm, zloc, SPLIT, NQT,
                               "z", use_collective)
    acc_z, d_z = _emit_stats_recv(nc, P, stats_z, negm, SPLIT, NQT, "z")

    # ---- GEMM2: out[k, d] = sum_q E[q, k] * V'[q, d] ------------
    # phase 0: q tiles [0, SPLIT) for ALL ki (only needs exchange A), each
    # [128, D] partial spilled to SBUF via the scalar engine; phase 1:
    # q tiles [SPLIT, NQT) with fresh PSUM groups, then DVE adds the spill
    # back in on the way out. Phase 0's ~27 us of PE work covers exchange
    # Z's collective latency.
    for ki in range(NKT):
        psg = PS.tile([128, S2L], F32, tag="ps", name=f"o{ki}")
        for db in range(2):
            for qi in range(SPLIT):
                nc.tensor.matmul(
                    psg[:, db * 512:(db + 1) * 512],
                    lhsT=e_sb[qi][:, ki * 128:(ki + 1) * 128],
                    rhs=v_sb[qi][:, db * 512:(db + 1) * 512],
                    start=(qi == 0),
                    stop=(qi == SPLIT - 1),
                )
        nc.scalar.copy(out=part[ki][:, :], in_=psg[:, :])
        if ki == 3:
            _emit_stats_finish(nc, P, acc_z, d_z, zloc, cs, SPLIT, NQT,
                               "z")
            for qj in range(SPLIT, NQT):
                nc.vector.tensor_scalar_mul(
                    out=v_sb[qj][:, 0:512], in0=v_sb[qj][:, 0:512],
                    scalar1=cs[:, qj:qj + 1])
            for qj in range(SPLIT, NQT):
                nc.vector.tensor_scalar_mul(
                    out=v_sb[qj][:, 512:D], in0=v_sb[qj][:, 512:D],
                    scalar1=cs[:, qj:qj + 1])
    for ki in range(NKT):
        final = ki == NKT - 1
        ot = OST.tile([128, D], F32, tag="ot", name=f"ot{ki}")
        if final:
            # the last ki accumulates into four independent 256-column PSUM
            # tiles: each quarter's add + store pipelines under the next
            # quarter's matmuls, so only the final 256-column add + store
            # trail the last matmul
            psq = [PS.tile([128, 256], F32, tag="ps", name=f"oq{j}")
                   for j in range(4)]
            for j in range(4):
                for qi in range(SPLIT, NQT):
                    nc.tensor.matmul(
                        psq[j][:, :],
                        lhsT=e_sb[qi][:, ki * 128:(ki + 1) * 128],
                        rhs=v_sb[qi][:, j * 256:(j + 1) * 256],
                        start=(qi == SPLIT),
                        stop=(qi == NQT - 1),
                    )
                nc.vector.tensor_tensor(
                    out=ot[:, j * 256:(j + 1) * 256],
                    in0=psq[j][:, :],
                    in1=part[ki][:, j * 256:(j + 1) * 256],
                    op=mybir.AluOpType.add)
                nc.sync.dma_start(
                    out=out[ki * 128:(ki + 1) * 128, j * 256:(j + 1) * 256],
                    in_=ot[:, j * 256:(j + 1) * 256])
            continue
        psg = PS.tile([128, S2L], F32, tag="ps", name=f"o{ki}b")
        for db in range(2):
            for qi in range(SPLIT, NQT):
                nc.tensor.matmul(
                    psg[:, db * 512:(db + 1) * 512],
                    lhsT=e_sb[qi][:, ki * 128:(ki + 1) * 128],
                    rhs=v_sb[qi][:, db * 512:(db + 1) * 512],
                    start=(qi == SPLIT),
                    stop=(qi == NQT - 1),
                )
        if not final:
            nc.vector.tensor_tensor(out=ot[:, :], in0=psg[:, :],
                                    in1=part[ki][:, :],
                                    op=mybir.AluOpType.add)
            nc.sync.dma_start(out=out[ki * 128:(ki + 1) * 128, :],
                              in_=ot[:, :])


def _build_kernel(nc, qT, kT, enc, res, sel, out, reps=1,
                  use_collective=True):
    tc = tile.TileContext(nc)
    with tc:
        with (
            tc.tile_pool(name="persist", bufs=1) as P,
            tc.tile_pool(name="stage", bufs=6) as ST,
            tc.tile_pool(name="psum", bufs=4, space="PSUM") as PS,
            tc.tile_pool(name="outst", bufs=4) as OST,
            tc.tile_pool(name="dram", bufs=1, space="DRAM") as DR,
        ):
            pools = (P, ST, PS, OST, DR)
            for _ in range(reps):
                _emit_body(nc, tc, pools, qT, kT, enc, res, sel, out,
                           use_collective)
    return nc


def build(reps=1, use_collective=True):
    nc = bacc.Bacc("TRN2", target_bir_lowering=False, debug=False,
                   num_devices=N_CORES)
    qT = nc.dram_tensor("qT", [D, S], FP16, kind="ExternalInput").ap()
    kT = nc.dram_tensor("kT", [D, S2L], FP16, kind="ExternalInput").ap()
    enc = nc.dram_tensor("enc", [S, D], FP16, kind="ExternalInput").ap()
    res = nc.dram_tensor("res", [S, D], FP16, kind="ExternalInput").ap()
    sel = nc.dram_tensor("sel", [128, 2], F32, kind="ExternalInput").ap()
    out = nc.dram_tensor("out", [S2L, D], F32, kind="ExternalOutput").ap()
    _build_kernel(nc, qT, kT, enc, res, sel, out, reps=reps,
                  use_collective=use_collective)
    nc.compile()
    return nc


def make_in_maps(enc_outputs, atten_outputs, enc_residual):
    enc_outputs = np.asarray(enc_outputs, dtype=np.float32)
    atten_outputs = np.asarray(atten_outputs, dtype=np.float32)
    enc_residual = np.asarray(enc_residual, dtype=np.float32)
    enc16 = enc_outputs.astype(np.float16)
    att16 = atten_outputs.astype(np.float16)
    res16 = enc_residual.astype(np.float16)
    in_maps = []
    for core in range(N_CORES):
        b, half = core // 2, core % 2
        sel = np.zeros((128, 2), np.float32)
        sel[:, (core & 1) ^ 1] = 1.0
        in_maps.append({
            "qT": np.ascontiguousarray(enc16[b].T),
            "kT": np.ascontiguousarray(att16[b, half * S2L:(half + 1) * S2L, :].T),
            "enc": enc16[b],
            "res": res16[b],
            "sel": sel,
        })
    return in_maps


def assemble(results):
    out = np.empty((B, S, D), np.float32)
    for core in range(N_CORES):
        b, half = core // 2, core % 2
        out[b, half * S2L:(half + 1) * S2L, :] = results[core]["out"]
    return out


_NC = None


def kernel(enc_outputs, atten_outputs, enc_residual):
    global _NC
    if _NC is None:
        _NC = build()
    in_maps = make_in_maps(enc_outputs, atten_outputs, enc_residual)
    last_err = None
    for _attempt in range(3):
        try:
            res = run_bass_kernel_spmd(_NC, in_maps,
                                       core_ids=list(range(N_CORES)))
            return assemble(res.results)
        except Exception as e:  # transient device/tunnel errors -- retry
            last_err = e
    raise last_err
